# revision 8
# baseline (speedup 1.0000x reference)
"""Trainium2 Bass kernel for nn_Attention_39865886442202 (sparse periodic local attention).

Design (v2):
  - Data-parallel over batch B=8 across 8 NeuronCores (one batch element per core).
  - Mask is periodic with period w=128: visibility depends only on (query residue p,
    key residue c) with an 11-wide clamped band.  Tokens are regrouped by residue
    (grouped col g = p*16 + t for token n = 128*t + p) straight from HBM via DMA
    access patterns, so the whole kernel works in "grouped" space with zero
    strided on-chip copies.
  - Scores in ST layout (keys on partitions, queries on free dim), query tiles of
    8 residues vs 3 aligned key chunks, with per-residue trimming (288 score cols
    per (tile, head) instead of 384).
  - Additive mask applied by ONE rank-8 matmul per (tile, head): stationary
    key-residue one-hot x per-tile mask-value rows.
  - exp() without max-subtraction (scores provably tiny), batched 4 heads per
    ScalarE activation instruction.
  - AV computed transposed (V stationary [keys, 32voc], exp(S) moving) which
    yields attnout^T directly -- no output transposes.  Softmax denominators via
    all-ones stationary matmuls (replicated across the 32 voc rows).  4 heads
    packed in PE column groups.
  - AV / Z / final-projection outputs live in the spare regions of the score
    PSUM banks (scores use 288 of each 512-col bank range), so two 4-bank score
    tiles cover all of PSUM with full double buffering.
"""

import math

import ml_dtypes
import numpy as np

import concourse.bass as bass
import concourse.mybir as mybir
import concourse.tile as tile
from concourse import bacc, bass_utils

DIM = 256
NUM_HEADS = 8
HEAD_DIM = 32
SCALE = HEAD_DIM ** (-0.5)
B = 8
N = 2048
W = 128
T = 16            # token blocks of 128 (and residue tiles of 8)
NEG = -30000.0
AV_OFF = 288      # av region inside psS bank 0 spare [288, 416)
Z_OFF = 512 + 288  # z region inside psS bank 1 spare [800, 928)
PRJ_OFF0 = 1024 + 288  # proj oc 0:128 in bank 2 spare
PRJ_OFF1 = 1536 + 288  # proj oc 128:256 in bank 3 spare

_CACHE = {}
LAST_EXEC_NS = None


def _window(p):
    """Valid key residues [lo, hi) for query residue p (from the torch mask)."""
    if p <= 5:
        return (0, 11)
    if p >= 122:
        return (117, 128)
    return (p - 5, p + 6)


def _blocks(k):
    """Score blocks for query tile k: list of (key chunk cg, rlo, rhi)."""
    if k == 0:
        return [(0, 0, 8), (1, 0, 8)]
    if k == 15:
        return [(14, 0, 8), (15, 0, 8)]
    return [(k - 1, 0, 5), (k, 0, 8), (k + 1, 3, 8)]


def _fd(k):
    return sum(16 * (rhi - rlo) for _, rlo, rhi in _blocks(k))


def _koffs():
    offs = []
    o = 0
    for k in range(T):
        offs.append(o)
        o += _fd(k)
    return offs, o


def _build_consts():
    bf = ml_dtypes.bfloat16
    # key-residue one-hot, replicated at 4 partition bases
    aone = np.zeros((128, 128), dtype=np.float32)
    for g in range(4):
        for j in range(8):
            aone[32 * g + j, 16 * j:16 * (j + 1)] = 1.0
    # mask values: per tile k, per block, per query residue
    koffs, mbw = _koffs()
    maskb = np.zeros((128, mbw), dtype=np.float32)
    for k in range(T):
        boff = koffs[k]
        for cg, rlo, rhi in _blocks(k):
            for ri in range(rhi - rlo):
                r = rlo + ri
                lo, hi = _window(8 * k + r)
                for j in range(8):
                    val = 0.0 if lo <= 8 * cg + j < hi else NEG
                    col = boff + 16 * ri
                    for g in range(4):
                        maskb[32 * g + j, col:col + 16] = val
            boff += 16 * (rhi - rlo)
    ident = np.eye(128, dtype=np.float32)
    return aone.astype(bf), maskb.astype(bf), ident.astype(bf)


def _build_program():
    koffs, mbw = _koffs()
    nc = bacc.Bacc(None, target_bir_lowering=False)
    f32 = mybir.dt.float32
    bf16 = mybir.dt.bfloat16

    x_in = nc.declare_dram_parameter("x", [N, DIM], f32, isOutput=False)
    wqkv_in = nc.declare_dram_parameter("wqkv", [DIM, 3 * DIM], f32, isOutput=False)
    wproj_in = nc.declare_dram_parameter("wproj", [DIM, DIM], f32, isOutput=False)
    bproj_in = nc.declare_dram_parameter("bproj", [DIM], f32, isOutput=False)
    aone_in = nc.declare_dram_parameter("aone", [128, 128], bf16, isOutput=False)
    maskb_in = nc.declare_dram_parameter("maskb", [128, mbw], bf16, isOutput=False)
    ident_in = nc.declare_dram_parameter("ident", [128, 128], bf16, isOutput=False)
    out_ext = nc.declare_dram_parameter("out", [N, DIM], f32, isOutput=True)

    # grouped view of x / out: token n = 128*t + 8*pm + pl -> chunk pm, row pl*16+t
    # (pl, t) iterate in dest-partition order; DMA matches flattened order.
    xg = x_in.rearrange("(t pm pl) d -> pl t pm d", pm=16, pl=8)
    outg = out_ext.rearrange("(t pm pl) d -> pl t pm d", pm=16, pl=8)

    with tile.TileContext(nc) as tc:
        with (
            tc.tile_pool(name="singles", bufs=1) as singles,
            tc.tile_pool(name="sbw", bufs=3) as sbw,
            tc.tile_pool(name="sbz", bufs=4) as sbz,
            tc.tile_pool(name="sbo", bufs=4) as sbo,
        ):
            # ---- constants / weights ----
            aone_sb = singles.tile([128, 128], bf16)
            nc.sync.dma_start(out=aone_sb, in_=aone_in[:, :])
            maskb_sb = singles.tile([128, mbw], bf16)
            nc.sync.dma_start(out=maskb_sb, in_=maskb_in[:, :])
            ident_sb = singles.tile([128, 128], bf16)
            nc.sync.dma_start(out=ident_sb, in_=ident_in[:, :])

            wqkv_sb = []
            for dc in range(2):
                t_ = singles.tile([128, 3 * DIM], bf16, tag=f"wqkv{dc}")
                nc.gpsimd.dma_start(out=t_, in_=wqkv_in[128 * dc:128 * (dc + 1), :])
                wqkv_sb.append(t_)
            wproj_sb = []
            for fc in range(2):
                t_ = singles.tile([128, DIM], bf16, tag=f"wproj{fc}")
                nc.gpsimd.dma_start(out=t_, in_=wproj_in[128 * fc:128 * (fc + 1), :])
                wproj_sb.append(t_)
            biasrow = singles.tile([1, DIM], bf16)
            bp = bproj_in[:]
            nc.gpsimd.dma_start(
                out=biasrow,
                in_=bass.AP(tensor=bp.tensor, offset=bp.offset, ap=[[0, 1], [1, DIM]]),
            )
            ones1 = singles.tile([1, 128], bf16)
            nc.gpsimd.memset(ones1, 1.0)
            onesT = singles.tile([128, 32], bf16)
            nc.gpsimd.memset(onesT, 1.0)

            # ---- x load (grouped rows), 4 DMAs of 4 chunks each ----
            xbf = singles.tile([128, 16 * DIM], bf16)
            xbf3 = xbf.rearrange("q (m d) -> q m d", m=16)
            for m in range(16):
                nc.gpsimd.dma_start(
                    out=xbf3[:, m, :],
                    in_=xg[:, :, m, :],
                )

            # ---- persistent SBUF tensors ----
            xTg = [singles.tile([128, N], bf16, name=f"xTg{dc}", tag=f"xTg{dc}")
                   for dc in range(2)]
            qT = [singles.tile([128, N], bf16, name=f"qT{g}", tag=f"qT{g}")
                  for g in range(2)]
            kT = [singles.tile([128, N], bf16, name=f"kT{g}", tag=f"kT{g}")
                  for g in range(2)]
            vsb = singles.tile([128, 16 * DIM], bf16)
            aoT = [singles.tile([128, N], bf16, name=f"aoT{g}", tag=f"aoT{g}")
                   for g in range(2)]

            # ---- phase A: transpose + QKV projections ----
            with (
                tc.tile_pool(name="pstp", bufs=3, space="PSUM") as pstp,
                tc.tile_pool(name="pspj", bufs=2, space="PSUM") as pspj,
            ):
                for dc in range(2):
                    for mg in range(4):
                        tp = pstp.tile([128, 512], bf16, tag="tp")
                        for mi in range(4):
                            m = 4 * mg + mi
                            nc.tensor.transpose(
                                tp[:, 128 * mi:128 * (mi + 1)],
                                xbf[:, 256 * m + 128 * dc:256 * m + 128 * (dc + 1)],
                                ident_sb,
                            )
                        nc.vector.tensor_copy(
                            xTg[dc][:, 512 * mg:512 * (mg + 1)], tp
                        )

                # Q/K projections: oc4 0,1 -> Q head groups; 2,3 -> K
                for oc4 in range(4):
                    dest = (qT[0], qT[1], kT[0], kT[1])[oc4]
                    for half in range(2):
                        ps = pspj.tile([128, 1024], f32, tag="pj")
                        for nf in range(2):
                            for dc in range(2):
                                nc.tensor.matmul(
                                    ps[:, 512 * nf:512 * (nf + 1)],
                                    lhsT=wqkv_sb[dc][:, 128 * oc4:128 * (oc4 + 1)],
                                    rhs=xTg[dc][:, 1024 * half + 512 * nf:
                                                1024 * half + 512 * (nf + 1)],
                                    start=(dc == 0), stop=(dc == 1),
                                )
                        if oc4 % 2 == 0:
                            nc.vector.tensor_copy(
                                dest[:, 1024 * half:1024 * (half + 1)], ps)
                        else:
                            nc.scalar.copy(
                                dest[:, 1024 * half:1024 * (half + 1)], ps)

                # V projection (keys stationary from xTg)
                for mq in range(4):
                    ps = pspj.tile([128, 1024], f32, tag="pj")
                    for mi in range(4):
                        m = 4 * mq + mi
                        for dc in range(2):
                            nc.tensor.matmul(
                                ps[:, 256 * mi:256 * (mi + 1)],
                                lhsT=xTg[dc][:, 128 * m:128 * (m + 1)],
                                rhs=wqkv_sb[dc][:, 2 * DIM:3 * DIM],
                                start=(dc == 0), stop=(dc == 1),
                            )
                    nc.vector.tensor_copy(vsb[:, 1024 * mq:1024 * (mq + 1)], ps)

            # ---- phase B: attention (+ fused final projection) ----
            with tc.tile_pool(name="psb", bufs=2, space="PSUM") as psb:
                for k in range(T):
                    blocks = _blocks(k)
                    fdk = _fd(k)
                    nb = len(blocks)
                    for grp in range(2):
                        qTg, kTg = qT[grp], kT[grp]
                        psS = psb.tile([128, 2048], f32, tag="psS")
                        ps3 = psS.rearrange("p (h c) -> p h c", h=4)
                        for hh in range(4):
                            base = 32 * hh
                            boff = 0
                            for bi, (cg, rlo, rhi) in enumerate(blocks):
                                nq = 16 * (rhi - rlo)
                                nc.tensor.matmul(
                                    ps3[:, hh, boff:boff + nq],
                                    lhsT=kTg[base:base + 32,
                                             128 * cg:128 * (cg + 1)],
                                    rhs=qTg[base:base + 32,
                                            128 * k + 16 * rlo:128 * k + 16 * rhi],
                                    start=(bi == 0), stop=False,
                                    tile_position=(base, 0),
                                )
                                boff += nq
                            nc.tensor.matmul(
                                ps3[:, hh, 0:fdk],
                                lhsT=aone_sb[base:base + 8, :],
                                rhs=maskb_sb[base:base + 8,
                                             koffs[k]:koffs[k] + fdk],
                                start=False, stop=True,
                                tile_position=(base, 0),
                            )
                        ptil = sbw.tile([128, 4 * 288], bf16, tag="ptil")
                        pt3 = ptil.rearrange("p (h c) -> p h c", h=4)
                        nc.scalar.activation(
                            pt3[:, :, 0:fdk], ps3[:, :, 0:fdk],
                            mybir.ActivationFunctionType.Exp,
                        )
                        # AV (transposed) + Z, 4 heads packed in column groups
                        for hh in range(4):
                            h = 4 * grp + hh
                            boff = 0
                            for bi, (cg, rlo, rhi) in enumerate(blocks):
                                nq = 16 * (rhi - rlo)
                                nc.tensor.matmul(
                                    psS[32 * hh:32 * (hh + 1),
                                        AV_OFF + 16 * rlo:AV_OFF + 16 * rlo + nq],
                                    lhsT=vsb[:, 256 * cg + 32 * h:
                                             256 * cg + 32 * (h + 1)],
                                    rhs=pt3[:, hh, boff:boff + nq],
                                    start=False, stop=(bi == nb - 1),
                                    tile_position=(0, 32 * hh),
                                    skip_group_check=True,
                                )
                                nc.tensor.matmul(
                                    psS[32 * hh:32 * (hh + 1),
                                        Z_OFF + 16 * rlo:Z_OFF + 16 * rlo + nq],
                                    lhsT=onesT[:, :],
                                    rhs=pt3[:, hh, boff:boff + nq],
                                    start=False, stop=(bi == nb - 1),
                                    tile_position=(0, 32 * hh),
                                    skip_group_check=True,
                                )
                                boff += nq
                        zrec = sbz.tile([128, 128], f32, tag="zrec")
                        nc.vector.reciprocal(zrec, psS[:, Z_OFF:Z_OFF + 128])
                        nc.vector.tensor_mul(
                            aoT[grp][:, 128 * k:128 * (k + 1)],
                            psS[:, AV_OFF:AV_OFF + 128],
                            zrec,
                        )
                        if grp == 1:
                            # final projection for tile k into this slot's spare
                            for och in range(2):
                                poff = (PRJ_OFF0, PRJ_OFF1)[och]
                                for fc in range(2):
                                    nc.tensor.matmul(
                                        psS[:, poff:poff + 128],
                                        lhsT=aoT[fc][:, 128 * k:128 * (k + 1)],
                                        rhs=wproj_sb[fc][:, 128 * och:
                                                         128 * (och + 1)],
                                        start=False, stop=False,
                                        skip_group_check=True,
                                    )
                                nc.tensor.matmul(
                                    psS[:, poff:poff + 128],
                                    lhsT=ones1[:, :],
                                    rhs=biasrow[:, 128 * och:128 * (och + 1)],
                                    start=False, stop=True,
                                    skip_group_check=True,
                                )
                                osb = sbo.tile([128, 128], bf16, tag="osb")
                                nc.vector.tensor_copy(osb, psS[:, poff:poff + 128])
                                nc.gpsimd.dma_start(
                                    out=outg[:, :, k,
                                             128 * och:128 * (och + 1)],
                                    in_=osb,
                                )
    nc.finalize()
    return nc


def kernel(x, w, Wqkv, Wproj, bproj, **kw):
    global LAST_EXEC_NS
    assert int(w) == W
    x = np.asarray(x, dtype=np.float32)
    Wqkv = np.asarray(Wqkv, dtype=np.float32).copy()
    Wproj = np.asarray(Wproj, dtype=np.float32)
    bproj = np.asarray(bproj, dtype=np.float32)
    Wqkv[:, :DIM] = Wqkv[:, :DIM] * SCALE  # fold attention scale into Wq

    if "prog" not in _CACHE:
        _CACHE["prog"] = _build_program()
        _CACHE["consts"] = _build_consts()
    nc = _CACHE["prog"]
    aone, maskb, ident = _CACHE["consts"]

    core_ids = list(range(B))
    in_maps = []
    for b in range(B):
        in_maps.append({
            "x": np.ascontiguousarray(x[b]),
            "wqkv": Wqkv,
            "wproj": Wproj,
            "bproj": bproj,
            "aone": aone,
            "maskb": maskb,
            "ident": ident,
        })
    res = bass_utils.run_bass_kernel_spmd(nc, in_maps, core_ids)
    globals()["LAST_RES"] = res
    LAST_EXEC_NS = res.exec_time_ns
    out = np.stack([res.results[b]["out"] for b in range(B)], axis=0)
    return out.astype(np.float32)


# revision 12
# speedup vs baseline: 1.5857x; 1.5857x over previous
"""Trainium2 Bass kernel for nn_Attention_39865886442202 (sparse periodic local attention).

Design (v2):
  - Data-parallel over batch B=8 across 8 NeuronCores (one batch element per core).
  - Mask is periodic with period w=128: visibility depends only on (query residue p,
    key residue c) with an 11-wide clamped band.  Tokens are regrouped by residue
    (grouped col g = p*16 + t for token n = 128*t + p) straight from HBM via DMA
    access patterns, so the whole kernel works in "grouped" space with zero
    strided on-chip copies.
  - Scores in ST layout (keys on partitions, queries on free dim), query tiles of
    8 residues vs 3 aligned key chunks, with per-residue trimming (288 score cols
    per (tile, head) instead of 384).
  - Additive mask applied by ONE rank-8 matmul per (tile, head): stationary
    key-residue one-hot x per-tile mask-value rows.
  - exp() without max-subtraction (scores provably tiny), batched 4 heads per
    ScalarE activation instruction.
  - AV computed transposed (V stationary [keys, 32voc], exp(S) moving) which
    yields attnout^T directly -- no output transposes.  Softmax denominators via
    all-ones stationary matmuls (replicated across the 32 voc rows).  4 heads
    packed in PE column groups.
  - AV / Z / final-projection outputs live in the spare regions of the score
    PSUM banks (scores use 288 of each 512-col bank range), so two 4-bank score
    tiles cover all of PSUM with full double buffering.
"""

import math

import ml_dtypes
import numpy as np

import concourse.bass as bass
import concourse.mybir as mybir
import concourse.tile as tile
from concourse import bacc, bass_utils

DIM = 256
NUM_HEADS = 8
HEAD_DIM = 32
SCALE = HEAD_DIM ** (-0.5)
B = 8
N = 2048
W = 128
T = 16            # token blocks of 128 (and residue tiles of 8)
NEG = -30000.0
AV_OFF = 288      # av region inside psS bank 0 spare [288, 416)
Z_OFF = 512 + 288  # z region inside psS bank 1 spare [800, 928)
PRJ_OFF0 = 1024 + 288  # proj oc 0:128 in bank 2 spare
PRJ_OFF1 = 1536 + 288  # proj oc 128:256 in bank 3 spare

_CACHE = {}
LAST_EXEC_NS = None


def _window(p):
    """Valid key residues [lo, hi) for query residue p (from the torch mask)."""
    if p <= 5:
        return (0, 11)
    if p >= 122:
        return (117, 128)
    return (p - 5, p + 6)


def _blocks(k):
    """Score blocks for query tile k: list of (key chunk cg, rlo, rhi)."""
    if k == 0:
        return [(0, 0, 8), (1, 0, 8)]
    if k == 15:
        return [(14, 0, 8), (15, 0, 8)]
    return [(k - 1, 0, 5), (k, 0, 8), (k + 1, 3, 8)]


def _fd(k):
    return sum(16 * (rhi - rlo) for _, rlo, rhi in _blocks(k))


def _koffs():
    offs = []
    o = 0
    for k in range(T):
        offs.append(o)
        o += _fd(k)
    return offs, o


def _build_consts():
    bf = ml_dtypes.bfloat16
    # key-residue one-hot, replicated at 4 partition bases
    aone = np.zeros((128, 128), dtype=np.float32)
    for g in range(4):
        for j in range(8):
            aone[32 * g + j, 16 * j:16 * (j + 1)] = 1.0
    # mask values: per tile k, per block, per query residue
    koffs, mbw = _koffs()
    maskb = np.zeros((128, mbw), dtype=np.float32)
    for k in range(T):
        boff = koffs[k]
        for cg, rlo, rhi in _blocks(k):
            for ri in range(rhi - rlo):
                r = rlo + ri
                lo, hi = _window(8 * k + r)
                for j in range(8):
                    val = 0.0 if lo <= 8 * cg + j < hi else NEG
                    col = boff + 16 * ri
                    for g in range(4):
                        maskb[32 * g + j, col:col + 16] = val
            boff += 16 * (rhi - rlo)
    ident = np.eye(128, dtype=np.float32)
    return aone.astype(bf), maskb.astype(bf), ident.astype(bf)


def _build_program():
    koffs, mbw = _koffs()
    nc = bacc.Bacc(None, target_bir_lowering=False)
    f32 = mybir.dt.float32
    bf16 = mybir.dt.bfloat16

    x_in = nc.declare_dram_parameter("x", [N, DIM], f32, isOutput=False)
    wqkv_in = nc.declare_dram_parameter("wqkv", [DIM, 3 * DIM], f32, isOutput=False)
    wproj_in = nc.declare_dram_parameter("wproj", [DIM, DIM], f32, isOutput=False)
    bproj_in = nc.declare_dram_parameter("bproj", [DIM], f32, isOutput=False)
    aone_in = nc.declare_dram_parameter("aone", [128, 128], bf16, isOutput=False)
    maskb_in = nc.declare_dram_parameter("maskb", [128, mbw], bf16, isOutput=False)
    ident_in = nc.declare_dram_parameter("ident", [128, 128], bf16, isOutput=False)
    out_ext = nc.declare_dram_parameter("out", [N, DIM], f32, isOutput=True)

    # grouped view of x / out: token n = 128*t + 8*pm + pl -> chunk pm, row pl*16+t
    # (pl, t) iterate in dest-partition order; DMA matches flattened order.
    xg = x_in.rearrange("(t pm pl) d -> pl t pm d", pm=16, pl=8)
    outg = out_ext.rearrange("(t pm pl) d -> pl t pm d", pm=16, pl=8)

    with tile.TileContext(nc) as tc:
        with (
            tc.tile_pool(name="singles", bufs=1) as singles,
            tc.tile_pool(name="sbw", bufs=3) as sbw,
            tc.tile_pool(name="sbz", bufs=4) as sbz,
            tc.tile_pool(name="sbo", bufs=4) as sbo,
        ):
            # ---- constants / weights ----
            aone_sb = singles.tile([128, 128], bf16)
            nc.sync.dma_start(out=aone_sb, in_=aone_in[:, :])
            maskb_sb = singles.tile([128, mbw], bf16)
            nc.sync.dma_start(out=maskb_sb, in_=maskb_in[:, :])
            ident_sb = singles.tile([128, 128], bf16)
            nc.sync.dma_start(out=ident_sb, in_=ident_in[:, :])

            wqkv_sb = []
            for dc in range(2):
                t_ = singles.tile([128, 3 * DIM], bf16, tag=f"wqkv{dc}")
                nc.gpsimd.dma_start(out=t_, in_=wqkv_in[128 * dc:128 * (dc + 1), :])
                wqkv_sb.append(t_)
            wproj_sb = []
            for fc in range(2):
                t_ = singles.tile([128, DIM], bf16, tag=f"wproj{fc}")
                nc.gpsimd.dma_start(out=t_, in_=wproj_in[128 * fc:128 * (fc + 1), :])
                wproj_sb.append(t_)
            biasrow = singles.tile([1, DIM], bf16)
            bp = bproj_in[:]
            nc.gpsimd.dma_start(
                out=biasrow,
                in_=bass.AP(tensor=bp.tensor, offset=bp.offset, ap=[[0, 1], [1, DIM]]),
            )
            ones1 = singles.tile([1, 128], bf16)
            nc.gpsimd.memset(ones1, 1.0)
            onesT = singles.tile([128, 32], bf16)
            nc.gpsimd.memset(onesT, 1.0)

            # ---- x load (grouped rows) on HWDGE queues, then cast to bf16 ----
            xraw = singles.tile([128, 16 * DIM], f32)
            xraw3 = xraw.rearrange("q (m d) -> q m d", m=16)
            xbf = singles.tile([128, 16 * DIM], bf16)
            dmaq = [nc.sync, nc.scalar]
            for m in range(16):
                dmaq[m % 2].dma_start(out=xraw3[:, m, :], in_=xg[:, :, m, :])
            for mq in range(4):
                nc.vector.tensor_copy(
                    xbf[:, 1024 * mq:1024 * (mq + 1)],
                    xraw[:, 1024 * mq:1024 * (mq + 1)],
                )

            # ---- persistent SBUF tensors ----
            xTg = [singles.tile([128, N], bf16, name=f"xTg{dc}", tag=f"xTg{dc}")
                   for dc in range(2)]
            qT = [singles.tile([128, N], bf16, name=f"qT{g}", tag=f"qT{g}")
                  for g in range(2)]
            kT = [singles.tile([128, N], bf16, name=f"kT{g}", tag=f"kT{g}")
                  for g in range(2)]
            vsb = singles.tile([128, 16 * DIM], bf16)
            aoT = [singles.tile([128, N], bf16, name=f"aoT{g}", tag=f"aoT{g}")
                   for g in range(2)]

            # ---- phase A: transpose + QKV projections ----
            with (
                tc.tile_pool(name="pstp", bufs=3, space="PSUM") as pstp,
                tc.tile_pool(name="pspj", bufs=2, space="PSUM") as pspj,
            ):
                for dc in range(2):
                    for mg in range(4):
                        tp = pstp.tile([128, 512], bf16, tag="tp")
                        for mi in range(4):
                            m = 4 * mg + mi
                            nc.tensor.transpose(
                                tp[:, 128 * mi:128 * (mi + 1)],
                                xbf[:, 256 * m + 128 * dc:256 * m + 128 * (dc + 1)],
                                ident_sb,
                            )
                        nc.vector.tensor_copy(
                            xTg[dc][:, 512 * mg:512 * (mg + 1)], tp
                        )

                # Q/K projections: oc4 0,1 -> Q head groups; 2,3 -> K
                for oc4 in range(4):
                    dest = (qT[0], qT[1], kT[0], kT[1])[oc4]
                    for half in range(2):
                        ps = pspj.tile([128, 1024], f32, tag="pj")
                        for nf in range(2):
                            for dc in range(2):
                                nc.tensor.matmul(
                                    ps[:, 512 * nf:512 * (nf + 1)],
                                    lhsT=wqkv_sb[dc][:, 128 * oc4:128 * (oc4 + 1)],
                                    rhs=xTg[dc][:, 1024 * half + 512 * nf:
                                                1024 * half + 512 * (nf + 1)],
                                    start=(dc == 0), stop=(dc == 1),
                                )
                        if oc4 % 2 == 0:
                            nc.vector.tensor_copy(
                                dest[:, 1024 * half:1024 * (half + 1)], ps)
                        else:
                            nc.scalar.copy(
                                dest[:, 1024 * half:1024 * (half + 1)], ps)

                # V projection (keys stationary from xTg)
                for mq in range(4):
                    ps = pspj.tile([128, 1024], f32, tag="pj")
                    for mi in range(4):
                        m = 4 * mq + mi
                        for dc in range(2):
                            nc.tensor.matmul(
                                ps[:, 256 * mi:256 * (mi + 1)],
                                lhsT=xTg[dc][:, 128 * m:128 * (m + 1)],
                                rhs=wqkv_sb[dc][:, 2 * DIM:3 * DIM],
                                start=(dc == 0), stop=(dc == 1),
                            )
                    nc.vector.tensor_copy(vsb[:, 1024 * mq:1024 * (mq + 1)], ps)

            # ---- phase B: attention (+ fused final projection) ----
            # Software pipeline: round r emits scores+mask (PE) and exp (ACT);
            # the AV/Z/normalize/projection "tail" of round r-1 is emitted
            # after round r's scores so each engine's in-order stream overlaps
            # consecutive rounds instead of serializing on the chain.
            with tc.tile_pool(name="psb", bufs=2, space="PSUM") as psb:
                rounds = [(k, grp) for k in range(T) for grp in range(2)]
                state = {}

                def emit_head(r):
                    k, grp = rounds[r]
                    blocks = _blocks(k)
                    fdk = _fd(k)
                    qTg, kTg = qT[grp], kT[grp]
                    psS = psb.tile([128, 2048], f32, tag="psS", name=f"psS{r}")
                    ps3 = psS.rearrange("p (h c) -> p h c", h=4)
                    for hh in range(4):
                        base = 32 * hh
                        boff = 0
                        for bi, (cg, rlo, rhi) in enumerate(blocks):
                            nq = 16 * (rhi - rlo)
                            nc.tensor.matmul(
                                ps3[:, hh, boff:boff + nq],
                                lhsT=kTg[base:base + 32,
                                         128 * cg:128 * (cg + 1)],
                                rhs=qTg[base:base + 32,
                                        128 * k + 16 * rlo:128 * k + 16 * rhi],
                                start=(bi == 0), stop=False,
                                tile_position=(base, 0),
                            )
                            boff += nq
                        nc.tensor.matmul(
                            ps3[:, hh, 0:fdk],
                            lhsT=aone_sb[base:base + 8, :],
                            rhs=maskb_sb[base:base + 8,
                                         koffs[k]:koffs[k] + fdk],
                            start=False, stop=True,
                            tile_position=(base, 0),
                        )
                    ptil = sbw.tile([128, 4 * 288], bf16, tag="ptil",
                                    name=f"ptil{r}")
                    pt3 = ptil.rearrange("p (h c) -> p h c", h=4)
                    nc.scalar.activation(
                        pt3[:, :, 0:fdk], ps3[:, :, 0:fdk],
                        mybir.ActivationFunctionType.Exp,
                    )
                    state[r] = (psS, pt3)

                def emit_tail(r):
                    k, grp = rounds[r]
                    blocks = _blocks(k)
                    nb = len(blocks)
                    psS, pt3 = state.pop(r)
                    for hh in range(4):
                        h = 4 * grp + hh
                        boff = 0
                        for bi, (cg, rlo, rhi) in enumerate(blocks):
                            nq = 16 * (rhi - rlo)
                            nc.tensor.matmul(
                                psS[32 * hh:32 * (hh + 1),
                                    AV_OFF + 16 * rlo:AV_OFF + 16 * rlo + nq],
                                lhsT=vsb[:, 256 * cg + 32 * h:
                                         256 * cg + 32 * (h + 1)],
                                rhs=pt3[:, hh, boff:boff + nq],
                                start=False, stop=(bi == nb - 1),
                                tile_position=(0, 32 * hh),
                                skip_group_check=True,
                            )
                            nc.tensor.matmul(
                                psS[32 * hh:32 * (hh + 1),
                                    Z_OFF + 16 * rlo:Z_OFF + 16 * rlo + nq],
                                lhsT=onesT[:, :],
                                rhs=pt3[:, hh, boff:boff + nq],
                                start=False, stop=(bi == nb - 1),
                                tile_position=(0, 32 * hh),
                                skip_group_check=True,
                            )
                            boff += nq
                    zrec = sbz.tile([128, 128], f32, tag="zrec",
                                    name=f"zrec{r}")
                    nc.vector.reciprocal_approx_fast(
                        out=zrec, in_=psS[:, Z_OFF:Z_OFF + 128])
                    nc.vector.tensor_mul(
                        aoT[grp][:, 128 * k:128 * (k + 1)],
                        psS[:, AV_OFF:AV_OFF + 128],
                        zrec,
                    )
                    if grp == 1:
                        # final projection for tile k into this slot's spare
                        for och in range(2):
                            poff = (PRJ_OFF0, PRJ_OFF1)[och]
                            for fc in range(2):
                                nc.tensor.matmul(
                                    psS[:, poff:poff + 128],
                                    lhsT=aoT[fc][:, 128 * k:128 * (k + 1)],
                                    rhs=wproj_sb[fc][:, 128 * och:
                                                     128 * (och + 1)],
                                    start=False, stop=False,
                                    skip_group_check=True,
                                )
                            nc.tensor.matmul(
                                psS[:, poff:poff + 128],
                                lhsT=ones1[:, :],
                                rhs=biasrow[:, 128 * och:128 * (och + 1)],
                                start=False, stop=True,
                                skip_group_check=True,
                            )
                            osb = sbo.tile([128, 128], f32, tag="osb",
                                           name=f"osb{r}_{och}")
                            nc.vector.tensor_copy(osb, psS[:, poff:poff + 128])
                            (nc.sync if och == 0 else nc.scalar).dma_start(
                                out=outg[:, :, k, 128 * och:128 * (och + 1)],
                                in_=osb,
                            )

                for r in range(len(rounds)):
                    emit_head(r)
                    if r >= 1:
                        emit_tail(r - 1)
                emit_tail(len(rounds) - 1)
    nc.finalize()
    return nc


def kernel(x, w, Wqkv, Wproj, bproj, **kw):
    global LAST_EXEC_NS
    assert int(w) == W
    x = np.asarray(x, dtype=np.float32)
    Wqkv = np.asarray(Wqkv, dtype=np.float32).copy()
    Wproj = np.asarray(Wproj, dtype=np.float32)
    bproj = np.asarray(bproj, dtype=np.float32)
    Wqkv[:, :DIM] = Wqkv[:, :DIM] * SCALE  # fold attention scale into Wq

    if "prog" not in _CACHE:
        _CACHE["prog"] = _build_program()
        _CACHE["consts"] = _build_consts()
    nc = _CACHE["prog"]
    aone, maskb, ident = _CACHE["consts"]

    core_ids = list(range(B))
    in_maps = []
    for b in range(B):
        in_maps.append({
            "x": np.ascontiguousarray(x[b]),
            "wqkv": Wqkv,
            "wproj": Wproj,
            "bproj": bproj,
            "aone": aone,
            "maskb": maskb,
            "ident": ident,
        })
    res = bass_utils.run_bass_kernel_spmd(nc, in_maps, core_ids)
    globals()["LAST_RES"] = res
    LAST_EXEC_NS = res.exec_time_ns
    out = np.stack([res.results[b]["out"] for b in range(B)], axis=0)
    return out.astype(np.float32)
